# revision 13
# baseline (speedup 1.0000x reference)
"""Bass/Tile TRN2 kernel for nn_Attention_3264175145281.

Computes, for each batch row b:
    energy[s] = encoder_outputs[b, s, :] @ W[0, :512]   (+ const(b), dropped)
    weights   = softmax(energy)
    context   = weights @ encoder_outputs[b]

The reference adds `hidden @ W[0, 512:] + bias` to every energy[s]; that term
is constant along s, and softmax is shift-invariant, so the output does not
depend on it.  We therefore stream encoder_outputs exactly once per core.

Sharding: batch dim across 8 NeuronCores (4 rows each), W replicated.

v5 design, per-core engine budgets against the ~80us DMA floor (420 GB/s):
  - DVE: custom op MUL_CUMSUM_ANT = inclusive prefix sum of x*w.  One scan
    instruction covers 16 chunks (8192 elems/partition), writing ONLY the
    chunk-boundary running sums via a stride-0 output AP into a [P,17]
    tile (verified on HW).  8 scans/core = ~73us.  Chunk energies are the
    differences of adjacent boundary sums (GPSIMD, free).
  - PE: context matmuls per wave + the Z (sum-of-exp) matmuls.  The PE HAM
    clock gate idles the array at 1.2 GHz unless busy, which would make
    the PE the bottleneck; paced heater matmuls (chained to each DMA) plus
    a pre-heat burst hold it at 2.4 GHz.  PE emission is delayed one scan
    unit so heaters sit in front of data-dependent matmuls in the queue.
  - ScalarE: exp+rowsum waves and the 1/Z output scaling only.
  - First and last supergroups run as four 4-chunk scans to cut pipeline
    head/tail latency.
"""

import os
import sys

import numpy as np

for _p in ("/opt/trn_rl_repo", os.path.expanduser("~/.axon_site/_ro/trn_rl_repo")):
    if os.path.isdir(_p) and _p not in sys.path:
        sys.path.insert(0, _p)

from contextlib import ExitStack

import concourse.bacc as bacc
import concourse.bass as bass
import concourse.mybir as mybir
import concourse.tile as tile
from concourse.bass_utils import run_bass_kernel_spmd

# ---- custom DVE op: out[p,t] = cumsum_t(in0[p,t] * in1[p,t]) ---------------
import concourse.dve_ops as dve_ops
from concourse.dve_ops import DveOp
from concourse.dve_spec import AluOp as DveAluOp
from concourse.dve_spec import Spec as DveSpec
from concourse.dve_spec import Src0, Src1, lower as dve_lower, scan as dve_scan
from concourse.dve_uop import DveOpSpec


def _register_mul_cumsum() -> DveOp:
    name = "MUL_CUMSUM_ANT"
    if name in dve_ops._SUB_OPCODE_FOR_NAME:
        return next(op for op in dve_ops.OPS if op.name == name)
    spec = DveSpec(
        body=dve_scan(DveAluOp.ADD, Src0 * Src1),
        reference=lambda in0, in1, s0, s1, imm2: np.cumsum(
            in0.reshape(in0.shape[0], -1).astype(np.float32)
            * in1.reshape(in0.shape[0], -1),
            axis=-1,
            dtype=np.float32,
        ).reshape(in0.shape),
    )
    row = max(dve_ops._SUB_OPCODE_FOR_NAME.values()) + 1  # 17; rows 1..31 free
    dve_ops._SUB_OPCODE_FOR_NAME[name] = row
    shas = {}
    for ver in ("v3", "v4"):
        s = DveOpSpec(name=name, opcode=row, uops=dve_lower(spec, ver=ver), rd1_en=True)
        shas[ver] = s.sha(ver)
    op = DveOp(name, spec, subdim=False, uops_sha=shas)
    dve_ops.OPS.append(op)
    dve_ops.CUSTOM_DVE_SPECS[name] = spec
    return op


MUL_CUMSUM = _register_mul_cumsum()
# ---------------------------------------------------------------------------

B, S, ENC = 32, 4096, 512
NCORES = 8
B_LOC = B // NCORES          # 4 batch rows per core
P = 128                      # SBUF partitions
GRP = 4                      # chunks per 1 MiB DMA piece
SG = 16                      # chunks per supergroup (one gx tile, 4 MiB)
NSG = S // (P * SG)          # 2 supergroups per batch row
NCH = S // P                 # 32 chunks of 128 positions per row
PREHEAT = 6                  # PE warm-up matmuls before the pipeline
HEAT_PER_DMA = 2             # paced heater matmuls per 1 MiB DMA piece
F32 = mybir.dt.float32
F32R = mybir.dt.float32r     # 1 cyc/col on PE at N>=256 (vs 4 for fp32)


def build_program(n_b: int = B_LOC) -> bass.Bass:
    nc = bacc.Bacc("TRN2", target_bir_lowering=False, debug=False)

    x = nc.dram_tensor("x", [n_b, S, ENC], F32R, kind="ExternalInput").ap()
    wenc = nc.dram_tensor("wenc", [1, ENC], F32R, kind="ExternalInput").ap()
    out = nc.dram_tensor("out", [n_b, ENC], F32, kind="ExternalOutput").ap()

    with tile.TileContext(nc) as tc, ExitStack() as ctx:
        const_pool = ctx.enter_context(tc.tile_pool(name="const", bufs=1))
        gx_pool = ctx.enter_context(tc.tile_pool(name="gx", bufs=5))
        ends_pool = ctx.enter_context(tc.tile_pool(name="ends", bufs=6))
        stat_pool = ctx.enter_context(tc.tile_pool(name="stat", bufs=2))
        pt_pool = ctx.enter_context(tc.tile_pool(name="pt", bufs=2))
        rs_pool = ctx.enter_context(tc.tile_pool(name="rs", bufs=10))
        tail_pool = ctx.enter_context(tc.tile_pool(name="tailp", bufs=4))
        psum_pool = ctx.enter_context(tc.tile_pool(name="psum", bufs=3, space="PSUM"))
        hps_pool = ctx.enter_context(tc.tile_pool(name="hpsum", bufs=1, space="PSUM"))

        wb = const_pool.tile([P, ENC], F32R, tag="wb")
        nc.sync.dma_start(wb[:], wenc[:, :].broadcast_to([P, ENC]))
        ones = const_pool.tile([P, 1], F32, tag="ones")
        nc.gpsimd.memset(ones[:], 1.0)
        heat_psum = hps_pool.tile([1, ENC], F32, tag="heat")

        def heater(rhs):
            nc.tensor.matmul(
                heat_psum[:], ones[:].bitcast(F32R), rhs,
                start=True, stop=True, skip_group_check=True,
            )

        for _ in range(PREHEAT):
            heater(wb[:])

        # ---- per-row state ------------------------------------------------
        def new_row(b, n_waves):
            return {
                "b": b,
                "energy": stat_pool.tile([P, NCH], F32, tag="energy", name="energy"),
                "p_t": pt_pool.tile([P, NCH], F32R, tag="p", name="p"),
                "ctx": psum_pool.tile([1, ENC], F32, tag="ctx", name="ctxp"),
                "z": psum_pool.tile([1, 1], F32, tag="z", name="zp"),
                "wave_i": 0,
                "n_waves": n_waves,
                "rowsums": [],
            }

        def emit_wave_pe(r, gx, j0, n):
            """Z + context matmuls for chunks [j0, j0+n) of row r (PE only)."""
            w = r["wave_i"]; r["wave_i"] += 1
            nc.tensor.matmul(
                r["z"][:], r["rowsums"][w][:], ones[:],
                start=(w == 0), stop=(w == r["n_waves"] - 1),
            )
            for j in range(j0, j0 + n):
                nc.tensor.matmul(
                    r["ctx"][:], r["p_t"][:, j:j + 1], gx[:, j % SG, :],
                    start=(j == 0), stop=(j == NCH - 1),
                )

        def emit_scan_unit(r, gx, j0, n):
            """DVE scan + gpsimd diff + ScalarE exp for chunks [j0, j0+n)."""
            c0 = j0 % SG
            ends = ends_pool.tile([P, n + 1], F32, tag=f"ends{n}", name=f"ends{n}")
            nc.gpsimd.memset(ends[:, 0:1], 0.0)
            nc.vector._custom_dve(
                MUL_CUMSUM,
                out=ends[:, 1:n + 1].unsqueeze(2).broadcast_to([P, n, ENC]),
                in0=gx[:, c0:c0 + n, :].bitcast(F32),
                in1=wb[:].bitcast(F32).unsqueeze(1).broadcast_to([P, n, ENC]),
            )
            nc.gpsimd.tensor_tensor(
                r["energy"][:, j0:j0 + n], ends[:, 1:n + 1], ends[:, 0:n],
                mybir.AluOpType.subtract,
            )
            rowsum = rs_pool.tile([P, 1], F32, tag="rowsum")
            r["rowsums"].append(rowsum)
            nc.scalar.activation(
                r["p_t"][:, j0:j0 + n], r["energy"][:, j0:j0 + n],
                mybir.ActivationFunctionType.Exp,
                accum_out=rowsum[:],
            )

        def make_tail(r):
            def tail():
                rz = tail_pool.tile([1, 1], F32, tag="rz")
                nc.vector.reciprocal(rz[:], r["z"][:])
                ot = tail_pool.tile([1, ENC], F32, tag="ot")
                nc.scalar.activation(
                    ot[:], r["ctx"][:], mybir.ActivationFunctionType.Copy,
                    scale=rz[:],
                )
                nc.sync.dma_start(out[r["b"]:r["b"] + 1, :], ot[:])
            return tail

        # ---- flat unit list: (b, sg, row_chunk0, n_chunks) ----------------
        # first and last supergroup run as 4x 4-chunk scans (short head/tail)
        units = []
        for b in range(n_b):
            for sg in range(NSG):
                if (b == 0 and sg == 0) or (b == n_b - 1 and sg == NSG - 1):
                    for q in range(SG // GRP):
                        units.append((b, sg, sg * SG + q * GRP, GRP))
                else:
                    units.append((b, sg, sg * SG, SG))
        waves_per_row = {}
        for (b, sg, j0, n) in units:
            waves_per_row[b] = waves_per_row.get(b, 0) + 1

        pe_q = []      # PE wave emissions, delayed one unit
        tail_q = []    # (emit_at_unit_idx, closure)
        cur = None
        gx_tiles = {}

        for i, (b, sg, j0, n) in enumerate(units):
            if cur is None or cur["b"] != b:
                cur = new_row(b, waves_per_row[b])

            # 1 MiB DMA pieces + paced heaters chained on the fresh data
            if (b, sg) not in gx_tiles:
                gx_tiles[(b, sg)] = gx_pool.tile([P, SG, ENC], F32R, tag="gx", name="gx")
            gx = gx_tiles[(b, sg)]
            for q in range(n // GRP):
                c0 = (j0 % SG) + q * GRP
                s_lo = (sg * SG + c0) * P  # piece start position within row b
                src = x[b, s_lo:s_lo + P * GRP, :]
                nc.sync.dma_start(
                    gx[:, c0:c0 + GRP, :], src.rearrange("(p k) e -> p k e", p=P)
                )
                for _ in range(HEAT_PER_DMA):
                    heater(gx[:, c0, :])

            emit_scan_unit(cur, gx, j0, n)
            pe_q.append(
                lambda r=cur, g=gx, a=j0, m=n: emit_wave_pe(r, g, a, m)
            )
            if len(pe_q) > 1:
                pe_q.pop(0)()
            while tail_q and tail_q[0][0] <= i:
                tail_q.pop(0)[1]()
            if j0 + n == NCH:  # last unit of this row
                tail_q.append((i + 1, make_tail(cur)))

        while pe_q:
            pe_q.pop(0)()
        for _, fn in tail_q:
            fn()

    nc.compile()
    return nc


_CACHED_NC = None


def _get_nc() -> bass.Bass:
    global _CACHED_NC
    if _CACHED_NC is None:
        _CACHED_NC = build_program()
    return _CACHED_NC


def run(inputs: dict, trace: bool = False, **kw):
    """Shard inputs, run on 8 cores, return (full_output, BassKernelResults)."""
    x_full = np.ascontiguousarray(np.asarray(inputs["encoder_outputs"], dtype=np.float32))
    w_full = np.ascontiguousarray(np.asarray(inputs["W"], dtype=np.float32))
    wenc = np.ascontiguousarray(w_full[:, :ENC])

    nc = _get_nc()
    in_maps = [
        {"x": np.ascontiguousarray(x_full[c * B_LOC:(c + 1) * B_LOC]), "wenc": wenc}
        for c in range(NCORES)
    ]
    res = run_bass_kernel_spmd(nc, in_maps, list(range(NCORES)), trace=trace, **kw)
    out = np.concatenate([res.results[c]["out"] for c in range(NCORES)], axis=0)
    return out.astype(np.float32), res


def kernel(encoder_outputs, hidden, W, b):
    out, _ = run({"encoder_outputs": encoder_outputs, "W": W})
    return out
